# revision 22
# baseline (speedup 1.0000x reference)
"""Mixtral sparse MoE block on 8 TRN2 NeuronCores.

Strategy (expert-parallel, per sharding hint):
  - Router (tiny: 2048x1024 @ 1024x8 + softmax + top-2) runs on host as part
    of the sharding step; it determines which tokens go to which core.
  - Core e holds expert e's weights (w1/w2/w3) and receives the tokens
    routed to expert e (zero-padded to a static capacity C), pre-transposed.
  - Weights and activations are cast to bf16 on the host: halves HBM traffic
    (the memory roofline) and enables Fast Weight Load on the PE so the
    128x128 LDWEIGHTS hides behind the 512-col matmul stream. PSUM
    accumulation stays fp32.
  - Device computes hidT = silu(W1 x^T) * (W3 x^T); outT = W2 hidT -- the
    full SwiGLU MLP in transposed layout.
  - Host scales each expert output row by its routing weight and scatter-adds
    back into the [T, H] output. Tokens beyond the per-expert capacity C are
    handled exactly on the host (small: only load-imbalance overflow).

DMA plan: only 3 queues exist (gpsimd SW-DGE, sync/scalar HW-DGE), one per
engine that can issue DMAs, and queue rate depends on contiguous row length
(>=4KB rows needed for full rate). gpsimd is dedicated to the w13 stream
(stage-1 critical path); sync/scalar carry the two x halves, then sync takes
the w2 tiles -- paced one per ic via a data dependency so they cannot crowd
the critical early window -- and the stage-2 output tiles.

Shapes are hardcoded for the graded problem:
  hidden_states [1, 2048, 1024], gate_w [8, 1024],
  w1/w3 [8, 3584, 1024], w2 [8, 1024, 3584], fp32.
"""

import os

import numpy as np
import ml_dtypes

import concourse.bass as bass
import concourse.tile as tile
from concourse import mybir
from concourse.bass_utils import run_bass_kernel_spmd

E = 8          # experts == cores
TOP_K = 2
H = 1024       # hidden
I = 3584       # intermediate
T = 2048       # tokens
P = 128
NH = H // P    # 8
NI = I // P    # 28
C = 512        # per-expert token capacity; overflow tokens go to the host path

F32 = mybir.dt.float32
BF16 = mybir.dt.bfloat16
BF16_NP = ml_dtypes.bfloat16

_cache = {}


def _build_moe_mlp():
    """One-expert SwiGLU MLP, SPMD on 8 cores, bf16 in / fp32 accumulate.

    Inputs (per core, host pre-arranged, all bf16):
      xTb  [P, NH*C]       xTb[p, hc*C+c]      = x[c, hc*P+p]   (tokens^T)
      w13c [NI, P, 2*NH*P] w13c[ic, hp, hc*P+ip]        = w1[ic*P+ip, hc*P+hp]
                           w13c[ic, hp, NH*P + hc*P+ip] = w3[ic*P+ip, hc*P+hp]
      w2c  [NH, P, NI*P]   w2c[hc, ip, ic*P+hp] = w2[hc*P+hp, ic*P+ip]
    Output:
      outT [H, C] bf16 = ((silu(x@w1.T) * (x@w3.T)) @ w2.T)^T
    """
    nc = bass.Bass(use_seq_codegen=True)
    xTb = nc.declare_dram_parameter("xTb", [P, NH * C], BF16, isOutput=False)
    w13c = nc.declare_dram_parameter("w13c", [NI, P, 2 * NH * P], BF16, isOutput=False)
    w2c = nc.declare_dram_parameter("w2c", [NH, P, NI * P], BF16, isOutput=False)
    outT = nc.declare_dram_parameter("outT", [H, C], BF16, isOutput=True)

    with tile.TileContext(nc) as tc:
        with (
            tc.tile_pool(name="x_pool", bufs=1) as x_pool,
            tc.tile_pool(name="hid_pool", bufs=1) as hid_pool,
            tc.tile_pool(name="w13_pool", bufs=12) as w13_pool,
            tc.tile_pool(name="w2_pool", bufs=8) as w2_pool,
            tc.tile_pool(name="scr_pool", bufs=1) as scr_pool,
            tc.tile_pool(name="ps1", bufs=3, space="PSUM") as ps1,
            tc.tile_pool(name="ps3", bufs=3, space="PSUM") as ps3,
            tc.tile_pool(name="pso", bufs=2, space="PSUM") as pso,
            tc.tile_pool(name="act_pool", bufs=4) as act_pool,
            tc.tile_pool(name="out_pool", bufs=4) as out_pool,
        ):
            # ---- Stage 0. Every transfer is a fully contiguous DRAM
            # block (4KB+ rows) spread over the three queues:
            #   gpsimd: the whole w13 tile stream
            #   sync:   x first half, then paced w2 tiles, then outputs
            #   scalar: x second half, then silu evictions
            x_sb = x_pool.tile([P, NH * C], BF16, tag="x", name="x")
            XH = NH * C // 2  # 2048 cols (4KB rows) per half
            w13_first = w13_pool.tile([P, 2 * NH * P], BF16, tag="w13", name="w13_0")
            nc.gpsimd.dma_start(out=w13_first[:], in_=w13c[0])
            nc.sync.dma_start(out=x_sb[:, :XH], in_=xTb[:, :XH])
            nc.scalar.dma_start(out=x_sb[:, XH:], in_=xTb[:, XH:])

            w2_sb = [
                w2_pool.tile([P, NI * P], BF16, tag="w2", name=f"w2_{hc}")
                for hc in range(NH)
            ]

            # PE warm-up: dummy matmuls on a scratch tile with no DMA
            # deps. They fill the input-DMA wait (~8.5-14.5us: the first
            # transfer on each queue has ~5-6us of startup latency) and
            # push the PE HAM clock gate to 8/8 (~3.4us of sustained
            # activity), so the real stream starts at 2.4GHz, warm.
            scr = scr_pool.tile([P, C], BF16, tag="scr", name="scr")
            nc.vector.memset(scr[:], 0.0)
            for k in range(21):
                pw = pso.tile([P, C], F32, tag="po")
                nc.tensor.matmul(
                    pw[:], lhsT=scr[:, :P], rhs=scr[:], start=True, stop=True
                )

            # hidT [I, C] lives in SBUF (bf16) between the two stages.
            hid_sb = [
                hid_pool.tile([P, C], BF16, tag=f"hid{ic}", name=f"hid{ic}")
                for ic in range(NI)
            ]

            # ---- Stage 1: hidT[ic] = silu(p1) * p3, contracting over H.
            # The whole w13 stream rides gpsimd's queue in tile order; the
            # pool depth (8) is the prefetch window.
            for ic in range(NI):
                if ic == 0:
                    w13t = w13_first
                else:
                    w13t = w13_pool.tile([P, 2 * NH * P], BF16, tag="w13")
                    nc.gpsimd.dma_start(out=w13t[:], in_=w13c[ic])
                w1t = w13t[:, : NH * P]
                w3t = w13t[:, NH * P:]
                p1 = ps1.tile([P, C], F32, tag="p1")
                p3 = ps3.tile([P, C], F32, tag="p3")
                for hc in range(NH):
                    nc.tensor.matmul(
                        p1[:],
                        lhsT=w1t[:, bass.ts(hc, P)],
                        rhs=x_sb[:, bass.ds(hc * C, C)],
                        start=(hc == 0),
                        stop=(hc == NH - 1),
                    )
                for hc in range(NH):
                    nc.tensor.matmul(
                        p3[:],
                        lhsT=w3t[:, bass.ts(hc, P)],
                        rhs=x_sb[:, bass.ds(hc * C, C)],
                        start=(hc == 0),
                        stop=(hc == NH - 1),
                    )
                # Evict: ACT does silu(p1) -> bf16, DVE multiplies by p3
                # straight out of PSUM. The w2 prefetch must NOT run during
                # the x/w13 critical early window, and the scheduler ignores
                # program order, so pace it with a real data dependency:
                # after the mul of ic=1+hc, DVE stamps a 1-column sliver of
                # w2_sb[hc] (reading hid, so the stamp itself cannot be
                # hoisted); the full-tile DMA on sync write-after-write
                # depends on that sliver, so it issues one tile per ic.
                s1 = act_pool.tile([P, C], BF16, tag="s1")
                nc.scalar.activation(
                    s1[:], p1[:], mybir.ActivationFunctionType.Silu
                )
                nc.vector.tensor_mul(hid_sb[ic][:], s1[:], p3[:])
                if 1 <= ic < 1 + NH:
                    # The stamp READS hid (just written by the mul above),
                    # so the scheduler cannot hoist it; the w2 DMA then
                    # write-after-write depends on the stamped sliver.
                    hc = ic - 1
                    nc.vector.tensor_copy(
                        w2_sb[hc][:, :1], hid_sb[ic][:, :1]
                    )
                    nc.sync.dma_start(out=w2_sb[hc][:], in_=w2c[hc])

            # ---- Stage 2: outT[hc] = w2 @ hid, contracting over I.
            # The last hc runs as two half-column accumulation groups so its
            # first half is evicted and in flight while the second half is
            # still on the PE -- the kernel's final DMA is then half-length.
            for hc in range(NH):
                row = outT[hc * P:(hc + 1) * P, :]
                halves = 1 if hc < NH - 1 else 2
                cw = C // halves
                for h in range(halves):
                    cs = bass.ds(h * cw, cw)
                    po = pso.tile([P, C], F32, tag="po")
                    for ic in range(NI):
                        nc.tensor.matmul(
                            po[:, :cw],
                            lhsT=w2_sb[hc][:, bass.ts(ic, P)],
                            rhs=hid_sb[ic][:, cs],
                            start=(ic == 0),
                            stop=(ic == NI - 1),
                        )
                    ot = out_pool.tile([P, C], BF16, tag="ot")
                    nc.scalar.copy(ot[:, :cw], po[:, :cw])
                    if halves == 1:
                        eng = nc.sync if hc % 2 == 0 else nc.gpsimd
                        eng.dma_start(out=row, in_=ot[:])
                    else:
                        # partition-split keeps 2KB rows on both queues
                        nc.sync.dma_start(
                            out=row[: P // 2, cs], in_=ot[: P // 2, :cw]
                        )
                        nc.gpsimd.dma_start(
                            out=row[P // 2:, cs], in_=ot[P // 2:, :cw]
                        )
    _split_excess_waits(nc)
    return nc


def _split_excess_waits(nc, max_inline=1):
    """This walrus build rejects instructions carrying more than one inline
    sem wait ("Too many sync wait commands"). Move excess on_wait entries
    onto standalone InstEventSemaphore ops right before the instruction on
    the same engine (semantically identical: the engine stalls either way).
    """
    for blk in nc.m.functions[0].blocks:
        insts = blk.instructions
        out = []
        changed = False
        for inst in insts:
            si = inst.sync_info
            waits = list(si.on_wait) if si is not None and si.on_wait else []
            if len(waits) > max_inline and not isinstance(
                inst, mybir.InstEventSemaphore
            ):
                excess, keep = waits[:-max_inline], waits[-max_inline:]
                for k, w in enumerate(excess):
                    out.append(
                        mybir.InstEventSemaphore(
                            name=f"{inst.name}-evw{k}",
                            engine=inst.engine,
                            sync_info=mybir.SyncInfo(on_wait=[w], on_update=[]),
                        )
                    )
                inst.sync_info = mybir.SyncInfo(
                    on_wait=keep, on_update=list(si.on_update or [])
                )
                changed = True
            out.append(inst)
        if changed:
            blk.instructions = out


def _route(x, gate_w):
    """Replicate the reference router in f64-stable numpy: returns
    (top_idx [T,K], top_w [T,K]) with renormalized weights."""
    logits = x.astype(np.float64) @ gate_w.astype(np.float64).T  # [T, E]
    m = logits.max(axis=-1, keepdims=True)
    p = np.exp(logits - m)
    p /= p.sum(axis=-1, keepdims=True)
    # top-2, ties broken by lower index (matches jax.lax.top_k)
    order = np.argsort(-p, axis=-1, kind="stable")
    top_i = order[:, :TOP_K]
    top_p = np.take_along_axis(p, top_i, axis=-1)
    top_w = top_p / top_p.sum(axis=-1, keepdims=True)
    return top_i, top_w.astype(np.float32)


def kernel(hidden_states, gate_w, w1, w2, w3):
    b, s, h = hidden_states.shape
    x = np.ascontiguousarray(
        np.asarray(hidden_states, dtype=np.float32).reshape(-1, h)
    )
    gate_w = np.asarray(gate_w, dtype=np.float32)
    w1 = np.asarray(w1, dtype=np.float32)
    w2 = np.asarray(w2, dtype=np.float32)
    w3 = np.asarray(w3, dtype=np.float32)

    top_i, top_w = _route(x, gate_w)

    # token lists per expert
    expert_rows = [np.where((top_i == e).any(axis=1))[0] for e in range(E)]
    in_maps = []
    overflow = []  # (e, token_idx, weight) handled exactly on host
    gathers = []
    for e in range(E):
        rows = expert_rows[e]
        if len(rows) > C:
            keep = rows[:C]
            for t in rows[C:]:
                kk = np.where(top_i[t] == e)[0][0]
                overflow.append((e, int(t), float(top_w[t, kk])))
            rows = keep
        gathers.append(rows)
        xe = np.zeros((C, H), dtype=np.float32)
        xe[: len(rows)] = x[rows]
        # xTb[p, hc*C+c] = xe[c, hc*P+p]
        xTb = np.ascontiguousarray(
            xe.T.reshape(NH, P, C).transpose(1, 0, 2).reshape(P, NH * C)
        ).astype(BF16_NP)
        w1c = w1[e].reshape(NI, P, NH, P).transpose(0, 3, 2, 1).reshape(NI, P, NH * P)
        w3c = w3[e].reshape(NI, P, NH, P).transpose(0, 3, 2, 1).reshape(NI, P, NH * P)
        w13c = np.ascontiguousarray(
            np.concatenate([w1c, w3c], axis=2)
        ).astype(BF16_NP)
        w2c = np.ascontiguousarray(
            w2[e].reshape(NH, P, NI, P).transpose(0, 3, 2, 1).reshape(NH, P, NI * P)
        ).astype(BF16_NP)
        in_maps.append({"xTb": xTb, "w13c": w13c, "w2c": w2c})

    if "nc" not in _cache:
        _cache["nc"] = _build_moe_mlp()
    nc = _cache["nc"]

    res = run_bass_kernel_spmd(
        nc,
        in_maps,
        core_ids=list(range(E)),
        trace=bool(int(os.environ.get("MOE_TRACE", "0"))),
    )
    _cache["last_result"] = res

    out = np.zeros((T, H), dtype=np.float32)
    for e in range(E):
        rows = gathers[e]
        ye = np.ascontiguousarray(
            res.results[e]["outT"].T.astype(np.float32)
        )[: len(rows)]  # [n_e, H]
        # routing weight of expert e for each routed token
        kidx = (top_i[rows] == e).argmax(axis=1)
        wts = top_w[rows, kidx][:, None]
        np.add.at(out, rows, ye * wts)

    if overflow:
        from collections import defaultdict
        by_e = defaultdict(list)
        for e, t, wt in overflow:
            by_e[e].append((t, wt))
        for e, lst in by_e.items():
            ts = np.array([t for t, _ in lst])
            wts = np.array([w for _, w in lst], dtype=np.float32)[:, None]
            xb = x[ts]
            hid = _silu_np(xb @ w1[e].T) * (xb @ w3[e].T)
            np.add.at(out, ts, wts * (hid @ w2[e].T))

    return out.reshape(b, s, h)


def _silu_np(v):
    return v / (1.0 + np.exp(-v))


# revision 23
# speedup vs baseline: 1.0003x; 1.0003x over previous
"""Mixtral sparse MoE block on 8 TRN2 NeuronCores.

Strategy (expert-parallel, per sharding hint):
  - Router (tiny: 2048x1024 @ 1024x8 + softmax + top-2) runs on host as part
    of the sharding step; it determines which tokens go to which core.
  - Core e holds expert e's weights (w1/w2/w3) and receives the tokens
    routed to expert e (zero-padded to a static capacity C), pre-transposed.
  - Weights and activations are cast to bf16 on the host: halves HBM traffic
    (the memory roofline) and enables Fast Weight Load on the PE so the
    128x128 LDWEIGHTS hides behind the 512-col matmul stream. PSUM
    accumulation stays fp32.
  - Device computes hidT = silu(W1 x^T) * (W3 x^T); outT = W2 hidT -- the
    full SwiGLU MLP in transposed layout.
  - Host scales each expert output row by its routing weight and scatter-adds
    back into the [T, H] output. Tokens beyond the per-expert capacity C are
    handled exactly on the host (small: only load-imbalance overflow).

DMA plan: only 3 queues exist (gpsimd SW-DGE, sync/scalar HW-DGE), one per
engine that can issue DMAs, and queue rate depends on contiguous row length
(>=4KB rows needed for full rate). gpsimd is dedicated to the w13 stream
(stage-1 critical path); sync/scalar carry the two x halves, then sync takes
the w2 tiles -- paced one per ic via a data dependency so they cannot crowd
the critical early window -- and the stage-2 output tiles.

Shapes are hardcoded for the graded problem:
  hidden_states [1, 2048, 1024], gate_w [8, 1024],
  w1/w3 [8, 3584, 1024], w2 [8, 1024, 3584], fp32.
"""

import os

import numpy as np
import ml_dtypes

import concourse.bass as bass
import concourse.tile as tile
from concourse import mybir
from concourse.bass_utils import run_bass_kernel_spmd

E = 8          # experts == cores
TOP_K = 2
H = 1024       # hidden
I = 3584       # intermediate
T = 2048       # tokens
P = 128
NH = H // P    # 8
NI = I // P    # 28
C = 512        # per-expert token capacity; overflow tokens go to the host path

F32 = mybir.dt.float32
BF16 = mybir.dt.bfloat16
BF16_NP = ml_dtypes.bfloat16

_cache = {}


def _build_moe_mlp():
    """One-expert SwiGLU MLP, SPMD on 8 cores, bf16 in / fp32 accumulate.

    Inputs (per core, host pre-arranged, all bf16):
      xTb  [P, NH*C]       xTb[p, hc*C+c]      = x[c, hc*P+p]   (tokens^T)
      w13c [NI, P, 2*NH*P] w13c[ic, hp, hc*P+ip]        = w1[ic*P+ip, hc*P+hp]
                           w13c[ic, hp, NH*P + hc*P+ip] = w3[ic*P+ip, hc*P+hp]
      w2c  [NH, P, NI*P]   w2c[hc, ip, ic*P+hp] = w2[hc*P+hp, ic*P+ip]
    Output:
      outT [H, C] bf16 = ((silu(x@w1.T) * (x@w3.T)) @ w2.T)^T
    """
    nc = bass.Bass(use_seq_codegen=True)
    xTb = nc.declare_dram_parameter("xTb", [P, NH * C], BF16, isOutput=False)
    w13c = nc.declare_dram_parameter("w13c", [NI, P, 2 * NH * P], BF16, isOutput=False)
    w2c = nc.declare_dram_parameter("w2c", [NH, P, NI * P], BF16, isOutput=False)
    outT = nc.declare_dram_parameter("outT", [H, C], BF16, isOutput=True)

    with tile.TileContext(nc) as tc:
        with (
            tc.tile_pool(name="x_pool", bufs=1) as x_pool,
            tc.tile_pool(name="hid_pool", bufs=1) as hid_pool,
            tc.tile_pool(name="w13_pool", bufs=12) as w13_pool,
            tc.tile_pool(name="w2_pool", bufs=8) as w2_pool,
            tc.tile_pool(name="scr_pool", bufs=1) as scr_pool,
            tc.tile_pool(name="ps1", bufs=3, space="PSUM") as ps1,
            tc.tile_pool(name="ps3", bufs=3, space="PSUM") as ps3,
            tc.tile_pool(name="pso", bufs=2, space="PSUM") as pso,
            tc.tile_pool(name="act_pool", bufs=4) as act_pool,
            tc.tile_pool(name="out_pool", bufs=4) as out_pool,
        ):
            # ---- Stage 0. Every transfer is a fully contiguous DRAM
            # block (4KB+ rows) spread over the three queues:
            #   gpsimd: the whole w13 tile stream
            #   sync:   x first half, then paced w2 tiles, then outputs
            #   scalar: x second half, then silu evictions
            x_sb = x_pool.tile([P, NH * C], BF16, tag="x", name="x")
            XH = NH * C // 2  # 2048 cols (4KB rows) per half
            w13_first = w13_pool.tile([P, 2 * NH * P], BF16, tag="w13", name="w13_0")
            nc.gpsimd.dma_start(out=w13_first[:], in_=w13c[0])
            nc.sync.dma_start(out=x_sb[:, :XH], in_=xTb[:, :XH])
            nc.scalar.dma_start(out=x_sb[:, XH:], in_=xTb[:, XH:])

            w2_sb = [
                w2_pool.tile([P, NI * P], BF16, tag="w2", name=f"w2_{hc}")
                for hc in range(NH)
            ]

            # PE warm-up: dummy matmuls on a scratch tile with no DMA
            # deps. They fill the input-DMA wait (~8.5-14.5us: the first
            # transfer on each queue has ~5-6us of startup latency) and
            # push the PE HAM clock gate to 8/8 (~3.4us of sustained
            # activity), so the real stream starts at 2.4GHz, warm.
            scr = scr_pool.tile([P, C], BF16, tag="scr", name="scr")
            nc.vector.memset(scr[:], 0.0)
            for k in range(18):
                pw = pso.tile([P, C], F32, tag="po")
                nc.tensor.matmul(
                    pw[:], lhsT=scr[:, :P], rhs=scr[:], start=True, stop=True
                )

            # hidT [I, C] lives in SBUF (bf16) between the two stages.
            hid_sb = [
                hid_pool.tile([P, C], BF16, tag=f"hid{ic}", name=f"hid{ic}")
                for ic in range(NI)
            ]

            # ---- Stage 1: hidT[ic] = silu(p1) * p3, contracting over H.
            # The whole w13 stream rides gpsimd's queue in tile order; the
            # pool depth (8) is the prefetch window.
            for ic in range(NI):
                if ic == 0:
                    w13t = w13_first
                else:
                    w13t = w13_pool.tile([P, 2 * NH * P], BF16, tag="w13")
                    nc.gpsimd.dma_start(out=w13t[:], in_=w13c[ic])
                w1t = w13t[:, : NH * P]
                w3t = w13t[:, NH * P:]
                p1 = ps1.tile([P, C], F32, tag="p1")
                p3 = ps3.tile([P, C], F32, tag="p3")
                for hc in range(NH):
                    nc.tensor.matmul(
                        p1[:],
                        lhsT=w1t[:, bass.ts(hc, P)],
                        rhs=x_sb[:, bass.ds(hc * C, C)],
                        start=(hc == 0),
                        stop=(hc == NH - 1),
                    )
                for hc in range(NH):
                    nc.tensor.matmul(
                        p3[:],
                        lhsT=w3t[:, bass.ts(hc, P)],
                        rhs=x_sb[:, bass.ds(hc * C, C)],
                        start=(hc == 0),
                        stop=(hc == NH - 1),
                    )
                # Evict: ACT does silu(p1) -> bf16, DVE multiplies by p3
                # straight out of PSUM. The w2 prefetch must NOT run during
                # the x/w13 critical early window, and the scheduler ignores
                # program order, so pace it with a real data dependency:
                # after the mul of ic=1+hc, DVE stamps a 1-column sliver of
                # w2_sb[hc] (reading hid, so the stamp itself cannot be
                # hoisted); the full-tile DMA on sync write-after-write
                # depends on that sliver, so it issues one tile per ic.
                s1 = act_pool.tile([P, C], BF16, tag="s1")
                nc.scalar.activation(
                    s1[:], p1[:], mybir.ActivationFunctionType.Silu
                )
                nc.vector.tensor_mul(hid_sb[ic][:], s1[:], p3[:])
                if 1 <= ic < 1 + NH:
                    # The stamp READS hid (just written by the mul above),
                    # so the scheduler cannot hoist it; the w2 DMA then
                    # write-after-write depends on the stamped sliver.
                    hc = ic - 1
                    nc.vector.tensor_copy(
                        w2_sb[hc][:, :1], hid_sb[ic][:, :1]
                    )
                    nc.sync.dma_start(out=w2_sb[hc][:], in_=w2c[hc])

            # ---- Stage 2: outT[hc] = w2 @ hid, contracting over I.
            # The last hc runs as two half-column accumulation groups so its
            # first half is evicted and in flight while the second half is
            # still on the PE -- the kernel's final DMA is then half-length.
            for hc in range(NH):
                row = outT[hc * P:(hc + 1) * P, :]
                halves = 1 if hc < NH - 1 else 2
                cw = C // halves
                for h in range(halves):
                    cs = bass.ds(h * cw, cw)
                    po = pso.tile([P, C], F32, tag="po")
                    for ic in range(NI):
                        nc.tensor.matmul(
                            po[:, :cw],
                            lhsT=w2_sb[hc][:, bass.ts(ic, P)],
                            rhs=hid_sb[ic][:, cs],
                            start=(ic == 0),
                            stop=(ic == NI - 1),
                        )
                    ot = out_pool.tile([P, C], BF16, tag="ot")
                    nc.scalar.copy(ot[:, :cw], po[:, :cw])
                    if halves == 1:
                        eng = nc.sync if hc % 2 == 0 else nc.gpsimd
                        eng.dma_start(out=row, in_=ot[:])
                    else:
                        # partition-split keeps 2KB rows on both queues
                        nc.sync.dma_start(
                            out=row[: P // 2, cs], in_=ot[: P // 2, :cw]
                        )
                        nc.gpsimd.dma_start(
                            out=row[P // 2:, cs], in_=ot[P // 2:, :cw]
                        )
    _split_excess_waits(nc)
    return nc


def _split_excess_waits(nc, max_inline=1):
    """This walrus build rejects instructions carrying more than one inline
    sem wait ("Too many sync wait commands"). Move excess on_wait entries
    onto standalone InstEventSemaphore ops right before the instruction on
    the same engine (semantically identical: the engine stalls either way).
    """
    for blk in nc.m.functions[0].blocks:
        insts = blk.instructions
        out = []
        changed = False
        for inst in insts:
            si = inst.sync_info
            waits = list(si.on_wait) if si is not None and si.on_wait else []
            if len(waits) > max_inline and not isinstance(
                inst, mybir.InstEventSemaphore
            ):
                excess, keep = waits[:-max_inline], waits[-max_inline:]
                for k, w in enumerate(excess):
                    out.append(
                        mybir.InstEventSemaphore(
                            name=f"{inst.name}-evw{k}",
                            engine=inst.engine,
                            sync_info=mybir.SyncInfo(on_wait=[w], on_update=[]),
                        )
                    )
                inst.sync_info = mybir.SyncInfo(
                    on_wait=keep, on_update=list(si.on_update or [])
                )
                changed = True
            out.append(inst)
        if changed:
            blk.instructions = out


def _route(x, gate_w):
    """Replicate the reference router in f64-stable numpy: returns
    (top_idx [T,K], top_w [T,K]) with renormalized weights."""
    logits = x.astype(np.float64) @ gate_w.astype(np.float64).T  # [T, E]
    m = logits.max(axis=-1, keepdims=True)
    p = np.exp(logits - m)
    p /= p.sum(axis=-1, keepdims=True)
    # top-2, ties broken by lower index (matches jax.lax.top_k)
    order = np.argsort(-p, axis=-1, kind="stable")
    top_i = order[:, :TOP_K]
    top_p = np.take_along_axis(p, top_i, axis=-1)
    top_w = top_p / top_p.sum(axis=-1, keepdims=True)
    return top_i, top_w.astype(np.float32)


def kernel(hidden_states, gate_w, w1, w2, w3):
    b, s, h = hidden_states.shape
    x = np.ascontiguousarray(
        np.asarray(hidden_states, dtype=np.float32).reshape(-1, h)
    )
    gate_w = np.asarray(gate_w, dtype=np.float32)
    w1 = np.asarray(w1, dtype=np.float32)
    w2 = np.asarray(w2, dtype=np.float32)
    w3 = np.asarray(w3, dtype=np.float32)

    top_i, top_w = _route(x, gate_w)

    # token lists per expert
    expert_rows = [np.where((top_i == e).any(axis=1))[0] for e in range(E)]
    in_maps = []
    overflow = []  # (e, token_idx, weight) handled exactly on host
    gathers = []
    for e in range(E):
        rows = expert_rows[e]
        if len(rows) > C:
            keep = rows[:C]
            for t in rows[C:]:
                kk = np.where(top_i[t] == e)[0][0]
                overflow.append((e, int(t), float(top_w[t, kk])))
            rows = keep
        gathers.append(rows)
        xe = np.zeros((C, H), dtype=np.float32)
        xe[: len(rows)] = x[rows]
        # xTb[p, hc*C+c] = xe[c, hc*P+p]
        xTb = np.ascontiguousarray(
            xe.T.reshape(NH, P, C).transpose(1, 0, 2).reshape(P, NH * C)
        ).astype(BF16_NP)
        w1c = w1[e].reshape(NI, P, NH, P).transpose(0, 3, 2, 1).reshape(NI, P, NH * P)
        w3c = w3[e].reshape(NI, P, NH, P).transpose(0, 3, 2, 1).reshape(NI, P, NH * P)
        w13c = np.ascontiguousarray(
            np.concatenate([w1c, w3c], axis=2)
        ).astype(BF16_NP)
        w2c = np.ascontiguousarray(
            w2[e].reshape(NH, P, NI, P).transpose(0, 3, 2, 1).reshape(NH, P, NI * P)
        ).astype(BF16_NP)
        in_maps.append({"xTb": xTb, "w13c": w13c, "w2c": w2c})

    if "nc" not in _cache:
        _cache["nc"] = _build_moe_mlp()
    nc = _cache["nc"]

    res = run_bass_kernel_spmd(
        nc,
        in_maps,
        core_ids=list(range(E)),
        trace=bool(int(os.environ.get("MOE_TRACE", "0"))),
    )
    _cache["last_result"] = res

    out = np.zeros((T, H), dtype=np.float32)
    for e in range(E):
        rows = gathers[e]
        ye = np.ascontiguousarray(
            res.results[e]["outT"].T.astype(np.float32)
        )[: len(rows)]  # [n_e, H]
        # routing weight of expert e for each routed token
        kidx = (top_i[rows] == e).argmax(axis=1)
        wts = top_w[rows, kidx][:, None]
        np.add.at(out, rows, ye * wts)

    if overflow:
        from collections import defaultdict
        by_e = defaultdict(list)
        for e, t, wt in overflow:
            by_e[e].append((t, wt))
        for e, lst in by_e.items():
            ts = np.array([t for t, _ in lst])
            wts = np.array([w for _, w in lst], dtype=np.float32)[:, None]
            xb = x[ts]
            hid = _silu_np(xb @ w1[e].T) * (xb @ w3[e].T)
            np.add.at(out, ts, wts * (hid @ w2[e].T))

    return out.reshape(b, s, h)


def _silu_np(v):
    return v / (1.0 + np.exp(-v))
